# revision 1
# baseline (speedup 1.0000x reference)
"""Multi-head attention TRN2 kernel (8 NeuronCores, SPMD).

Problem: B=2, N=2048, D=1024, H=16 heads of dim 64, fp32, per-(b,h)
key-length masking (valid_len, length 32).

Sharding: batch*heads across 8 cores - core c handles batch b=c//4 and 4
heads ("slots", rank-aligned by valid_len so the SPMD trip counts stay
balanced).  Per core:

  phase P (projections, bf16 inputs to halve HBM traffic):
    K^T/Q^T = Wslice^T @ x^T   (head dims on partitions, positions free)
    V       = x^T-tiles as lhsT, Wv as rhs  (positions on partitions),
              copied into V1 = [V_j | 1] blocks per (key-tile, slot)
  phase A (attention, all-bf16 operands, f32 PSUM accumulate):
    S^T   = K^T.T @ Q^T per (slot, key-tile), batched in PSUM pairs
    P^T   = exp(S^T/8 + bias) on ScalarE - the valid_len mask is a
            per-partition bias column (0 / -30000), so no V masking ops;
            key-tile pairs that are uniformly valid/invalid across all
            cores share one exp instruction ([128,1024] batch); groups
            are emitted round-robin across slots so independent
            S->exp->PV chains keep the tensor engine dense
    acc   = V1.T @ P^T accumulated over key tiles (ones column gives the
            softmax denominator as row 64)
    normalize per slot right after its last PV: DVE bit-exact
    reciprocal, GpSimd partition broadcast, DVE scale
    out_partial = heads^T.T @ Wo_slice -> (2048, 1024) per core; each
    chunk's output projection is emitted a few groups into the NEXT
    chunk so its PSUM slots never starve the S-matmul pipeline
Host sums the 4 per-core partials of each batch element (the unshard for
the row-sharded Wo) and gathers.
"""
import sys
import numpy as np
from contextlib import ExitStack

sys.path.insert(0, "/opt/trn_rl_repo")

import concourse.bass as bass  # noqa: E402
from concourse import bacc, mybir  # noqa: E402
import concourse.tile as tile  # noqa: E402
from concourse.bass_utils import run_bass_kernel_spmd  # noqa: E402

F32 = mybir.dt.float32
F32R = mybir.dt.float32r
BF16 = mybir.dt.bfloat16
AF = mybir.ActivationFunctionType
NPBF16 = mybir.dt.np(BF16)

B, N, D, H = 2, 2048, 1024, 16
DH = 64
HPC = 4          # heads (slots) per core
NCORES = 8
QC = 512         # q chunk (matmul free dim)
NKT = N // 128   # 16 k tiles
NDC = D // 128   # 8 contraction chunks
MASK_BIAS = -30000.0

LAST_RESULTS = None  # BassKernelResults of the most recent run (for tooling)


def _build_program(trips, plans):
    """trips: 4 ints (k-tile count per slot); plans: per slot, list of
    (t0, ntiles) exp-batch groups covering range(trips[j])."""
    nc = bacc.Bacc("TRN2", target_bir_lowering=False, debug=False,
                   num_devices=NCORES)

    xTq = nc.dram_tensor("xTq", [D, N], BF16, kind="ExternalInput")
    xTk = nc.dram_tensor("xTk", [D, N], BF16, kind="ExternalInput")
    xTv = nc.dram_tensor("xTv", [D, N], BF16, kind="ExternalInput")
    wq = nc.dram_tensor("wq", [128, NDC * 256], BF16, kind="ExternalInput")
    wk = nc.dram_tensor("wk", [128, NDC * 256], BF16, kind="ExternalInput")
    wv = nc.dram_tensor("wv", [128, NDC * 256], BF16, kind="ExternalInput")
    wo = nc.dram_tensor("wo", [256, D], BF16, kind="ExternalInput")
    vmask = nc.dram_tensor("vmask", [128, HPC * NKT], F32, kind="ExternalInput")
    out = nc.dram_tensor("out", [N, D], BF16, kind="ExternalOutput")

    with tile.TileContext(nc) as tc:
        with ExitStack() as ctx:
            wpool = ctx.enter_context(tc.tile_pool(name="wpool", bufs=1))
            xpool = ctx.enter_context(tc.tile_pool(name="xpool", bufs=6))
            qkpool = ctx.enter_context(tc.tile_pool(name="qkpool", bufs=1))
            v1pool = ctx.enter_context(tc.tile_pool(name="v1pool", bufs=1))
            ptpool = ctx.enter_context(tc.tile_pool(name="ptpool", bufs=8))
            nrmpool = ctx.enter_context(tc.tile_pool(name="nrmpool", bufs=4))
            pbpool = ctx.enter_context(tc.tile_pool(name="pbpool", bufs=1))
            opool = ctx.enter_context(tc.tile_pool(name="opool", bufs=8))

            # only wk is needed before the first matmul; the other weight
            # loads are issued just before their consuming phase so they
            # don't delay the first xk chunks
            t_wk = wpool.tile([128, NDC * 256], BF16, tag="wk")
            t_wq = wpool.tile([128, NDC * 256], BF16, tag="wq")
            t_wv = wpool.tile([128, NDC * 256], BF16, tag="wv")
            t_wo = [wpool.tile([128, D], BF16, tag=f"wo{p}", name=f"t_wo{p}")
                    for p in range(2)]
            t_vm = wpool.tile([128, HPC * NKT], F32, tag="vm")

            # K^T/Q^T: [128 dims (2 slots), N] per slot-pair
            t_kT = [qkpool.tile([128, N], BF16, tag=f"kT{p}", name=f"t_kT{p}")
                    for p in range(2)]
            t_qT = [qkpool.tile([128, N], BF16, tag=f"qT{p}", name=f"t_qT{p}")
                    for p in range(2)]
            # V1: per key-tile t, 4 blocks of [V_j (64 cols) | ones (1 col)]
            t_v1 = v1pool.tile([128, NKT * HPC * 65], BF16, tag="v1")
            # normalized heads^T per slot pair: [128 dims, N]
            t_pb = [pbpool.tile([128, N], BF16, tag=f"pb{p}", name=f"t_pb{p}")
                    for p in range(2)]

            # ones columns of V1, one strided memset
            ones_ap = t_v1[:].rearrange(
                "p (b c) -> p b c", c=65)[:, :, 64:65]
            nc.vector.memset(ones_ap, 1.0)

            # ---- phase P: projections (K, Q, V) ----
            with tc.tile_pool(name="pp", bufs=8, space="PSUM") as pp:
                for si, (xin, wsb, dsts) in enumerate(
                        ((xTk, t_wk, t_kT), (xTq, t_wq, t_qT))):
                    accs = [pp.tile([128, QC], F32, tag="acc", name=f"acc_{i}")
                            for i in range(8)]
                    for c in range(NDC):
                        xt = xpool.tile([128, N], BF16, tag="xt")
                        nc.sync.dma_start(xt[:], xin[c * 128:(c + 1) * 128, :])
                        if si == 0:
                            # wk sliced per chunk: the first matmul only
                            # gates on 64KB of weights + one x chunk
                            nc.sync.dma_start(
                                t_wk[:, c * 256:(c + 1) * 256],
                                wk[:, c * 256:(c + 1) * 256])
                        for m in range(2):
                            for qq in range(4):
                                nc.tensor.matmul(
                                    accs[m * 4 + qq][:],
                                    wsb[:, c * 256 + m * 128:
                                        c * 256 + (m + 1) * 128],
                                    xt[:, qq * QC:(qq + 1) * QC],
                                    start=(c == 0), stop=(c == NDC - 1))
                    if si == 0:
                        nc.sync.dma_start(t_wq[:], wq[:])
                    else:
                        nc.sync.dma_start(t_wv[:], wv[:])
                    # drain PSUM->SBUF casts on BOTH ScalarE and DVE, q-chunk
                    # 0 first: the first attention S matmuls gate only on the
                    # chunk-0 casts, so phase A starts ~2.5us earlier
                    for n, i in enumerate((0, 4, 1, 5, 2, 6, 3, 7)):
                        dst = dsts[i // 4][:, (i % 4) * QC:(i % 4 + 1) * QC]
                        with nc.allow_low_precision(reason="f32r 4B"):
                            if n % 2 == 0:
                                nc.scalar.activation(dst, accs[i][:], AF.Copy)
                            else:
                                nc.vector.tensor_copy(dst, accs[i][:])
                nc.sync.dma_start(t_wo[0][:], wo[0:128, :])
                nc.sync.dma_start(t_wo[1][:], wo[128:256, :])
                nc.sync.dma_start(t_vm[:], vmask[:])
                # V projection: two half-column passes of 8 k-tiles
                for g in range(2):
                    accs = [pp.tile([128, 256], F32, tag="acc",
                                    name=f"accv_{i}") for i in range(8)]
                    for c in range(NDC):
                        xt = xpool.tile([128, 1024], BF16, tag="xtv")
                        nc.sync.dma_start(
                            xt[:], xTv[c * 128:(c + 1) * 128,
                                       g * 1024:(g + 1) * 1024])
                        for kt8 in range(8):
                            nc.tensor.matmul(
                                accs[kt8][:],
                                xt[:, kt8 * 128:(kt8 + 1) * 128],
                                t_wv[:, c * 256:(c + 1) * 256],
                                start=(c == 0), stop=(c == NDC - 1))
                    for kt8 in range(8):
                        t = g * 8 + kt8
                        # [128, 4, 64] strided copy: slot j -> V1 block
                        src = accs[kt8][:].rearrange("p (j c) -> p j c", c=64)
                        dst = t_v1[:, t * 260:(t + 1) * 260].rearrange(
                            "p (j c) -> p j c", c=65)[:, :, 0:64]
                        with nc.allow_low_precision(reason="f32r 4B"):
                            nc.vector.tensor_copy(dst, src)

            # ---- phase A: attention with fused output projection ----
            with tc.tile_pool(name="ap", bufs=1, space="PSUM") as ap:
                def emit_outproj(q, qts=None):
                    # output projection for q-tiles of chunk q; accumulate
                    # pair 1 (shorter-plan slots) first so the first o_ps
                    # matmul fires before pair 0 is normalized
                    if qts is None:
                        qts = range(q * (QC // 128), (q + 1) * (QC // 128))
                    for qt in qts:
                        ts = slice(qt * 128, (qt + 1) * 128)
                        stage = opool.tile([128, D], BF16, tag="ostage")
                        for ch in range(2):
                            o_ps = ap.tile([128, 512], F32, tag="sT", bufs=2)
                            for p2 in (1, 0):
                                nc.tensor.matmul(
                                    o_ps[:], t_pb[p2][:, ts],
                                    t_wo[p2][:, ch * 512:(ch + 1) * 512],
                                    start=(p2 == 1), stop=(p2 == 0))
                            with nc.allow_low_precision(reason="bf16 out"):
                                nc.vector.tensor_copy(
                                    stage[:, ch * 512:(ch + 1) * 512],
                                    o_ps[:])
                        nc.sync.dma_start(out[ts, :], stage[:])

                for q in range(N // QC):
                    qs = slice(q * QC, (q + 1) * QC)
                    accs2 = [ap.tile([65, QC], F32, tag="acc2", bufs=4,
                                     name=f"acc_{j}") for j in range(HPC)]
                    # round-robin the exp-batch groups across slots so the
                    # in-flight S->exp->PV chains are independent and PE
                    # always has a matmul ready while ScalarE runs exp
                    sched = []
                    for j in range(HPC):
                        for gi, g in enumerate(plans[j]):
                            sched.append((j, gi, g))
                    sched.sort(key=lambda x: (x[1], x[0]))
                    for gidx, (j, gi, (t0, nt)) in enumerate(sched):
                        if gidx == 4 and q > 0:
                            # previous chunk's output projection, emitted a
                            # few groups in so its PSUM slots don't block
                            # this chunk's S matmuls during the norm tail
                            emit_outproj(q - 1)
                        p, half = j // 2, j % 2
                        rows = slice(half * 64, (half + 1) * 64)
                        acc = accs2[j]
                        sT = ap.tile([128, nt * QC], F32, tag="sT", bufs=2)
                        for i in range(nt):
                            t = t0 + i
                            nc.tensor.matmul(
                                sT[:, i * QC:(i + 1) * QC],
                                t_kT[p][rows, t * 128:(t + 1) * 128],
                                t_qT[p][rows, qs],
                                start=True, stop=True)
                        pT = ptpool.tile([128, nt * QC], BF16, tag="pT")
                        nc.scalar.activation(
                            pT[:], sT[:], AF.Exp, scale=0.125,
                            bias=t_vm[:, j * NKT + t0: j * NKT + t0 + 1])
                        for i in range(nt):
                            t = t0 + i
                            base = (t * HPC + j) * 65
                            nc.tensor.matmul(
                                acc[:], t_v1[:, base: base + 65],
                                pT[:, i * QC:(i + 1) * QC],
                                start=(gi == 0 and i == 0),
                                stop=(gi == len(plans[j]) - 1 and i == nt - 1))
                        if gi == len(plans[j]) - 1:
                            # normalize this slot as soon as its last PV is
                            # in: 1/denom (DVE), partition-broadcast
                            # (GpSimd), scale (DVE)
                            r1 = nrmpool.tile([1, QC], F32, tag="r1")
                            nc.vector.reciprocal(r1[:], acc[64:65, :])
                            bc_sb = nrmpool.tile([64, QC], F32, tag="bc_sb")
                            nc.gpsimd.partition_broadcast(bc_sb[:], r1[:])
                            with nc.allow_low_precision(reason="f32r 4B"):
                                nc.vector.tensor_mul(
                                    t_pb[p][rows, qs], acc[0:64, :],
                                    bc_sb[:])
                emit_outproj(N // QC - 1)

    nc.finalize()
    return nc


def _make_plans(trips, vls_by_slot):
    """Greedy pair batching: (t, t+1) share one exp iff every core's vl is
    outside the open interval (128*t, 128*(t+2)) - then one bias column
    describes both tiles on every core."""
    plans = []
    for j in range(HPC):
        plan, t = [], 0
        while t < trips[j]:
            if t + 1 < trips[j] and all(
                    v <= 128 * t or v >= 128 * (t + 2)
                    for v in vls_by_slot[j]):
                plan.append((t, 2))
                t += 2
            else:
                plan.append((t, 1))
                t += 1
        plans.append(plan)
    return plans


def kernel(queries, keys, values, valid_len, Wq, Wk, Wv, Wo):
    global LAST_RESULTS
    queries = np.asarray(queries, dtype=np.float32)
    keys = np.asarray(keys, dtype=np.float32)
    values = np.asarray(values, dtype=np.float32)
    Wq = np.asarray(Wq, dtype=np.float32)
    Wk = np.asarray(Wk, dtype=np.float32)
    Wv = np.asarray(Wv, dtype=np.float32)
    Wo = np.asarray(Wo, dtype=np.float32)
    vl = np.asarray(valid_len).astype(np.int64).reshape(B * H)

    # rank-aligned slot assignment: per batch, heads sorted by vl desc;
    # slot j of the 4 cores of that batch takes ranks 4j..4j+3
    order = {}
    for b in range(B):
        idx = (np.argsort(-vl[b * H:(b + 1) * H], kind="stable") + b * H)
        for cg in range(4):
            order[b * 4 + cg] = [int(idx[4 * j + cg]) for j in range(HPC)]
    trips, vls_by_slot = [], []
    for j in range(HPC):
        vs = [int(vl[order[c][j]]) for c in range(NCORES)]
        vls_by_slot.append(vs)
        m = max(-(-v // 128) for v in vs)
        trips.append(max(1, min(NKT, m)))
    plans = _make_plans(trips, vls_by_slot)

    nc = _build_program(tuple(trips), plans)

    in_maps = []
    for c in range(NCORES):
        b = c // 4
        heads = order[c]
        cols = np.concatenate(
            [np.arange((h - b * H) * DH, (h - b * H + 1) * DH) for h in heads])

        def wlayout(w):
            return np.ascontiguousarray(
                w[:, cols].reshape(NDC, 128, 256).transpose(1, 0, 2)
                .reshape(128, NDC * 256).astype(NPBF16))

        vm = np.zeros((128, HPC * NKT), np.float32)
        for j, h in enumerate(heads):
            bias = np.where(np.arange(N) < vl[h], 0.0, MASK_BIAS)
            vm[:, j * NKT:(j + 1) * NKT] = bias.reshape(NKT, 128).T

        in_maps.append({
            "xTq": np.ascontiguousarray(queries[b].T.astype(NPBF16)),
            "xTk": np.ascontiguousarray(keys[b].T.astype(NPBF16)),
            "xTv": np.ascontiguousarray(values[b].T.astype(NPBF16)),
            "wq": wlayout(Wq),
            "wk": wlayout(Wk),
            "wv": wlayout(Wv),
            "wo": np.ascontiguousarray(Wo[cols, :]).astype(NPBF16),
            "vmask": vm,
        })

    LAST_RESULTS = run_bass_kernel_spmd(nc, in_maps, list(range(NCORES)))
    res = LAST_RESULTS.results

    out = np.zeros((B, N, D), np.float64)
    for c in range(NCORES):
        out[c // 4] += res[c]["out"].astype(np.float64)
    return out.astype(np.float32)



# revision 4
# speedup vs baseline: 1.2272x; 1.2272x over previous
"""Multi-head attention TRN2 kernel (8 NeuronCores, SPMD).

Problem: B=2, N=2048, D=1024, H=16 heads of dim 64, fp32, per-(b,h)
key-length masking (valid_len, length 32).

Sharding: batch*heads across 8 cores - core c handles batch b=c//4 and 4
heads ("slots", rank-aligned by valid_len so the SPMD trip counts stay
balanced).  Per core:

  phase P (projections, bf16 inputs to halve HBM traffic):
    K^T/Q^T = Wslice^T @ x^T   (head dims on partitions, positions free)
    V       = x^T-tiles as lhsT, Wv as rhs  (positions on partitions),
              copied into V1 = [V_j | ones(64)] blocks per (key-tile, slot)
              - the 64 replicated ones columns make the PV matmul emit the
              softmax denominator PRE-BROADCAST in PSUM rows 64:127
  phase A (attention, all-bf16 operands, f32 PSUM accumulate):
    flat round-robin over (slot, key-tile) items per 512-query chunk with
    a PV lookahead stagger: S(i) and exp(i) are emitted immediately, but
    PV(i-L) is emitted L items later, so a PV waiting on its exp never
    head-of-line-blocks the next S matmul in the PE's strict-FIFO queue
    (the previous interleave serialized S->exp->PV per group, leaving
    both PE and ScalarE ~50% idle and oscillating the HAM clock gate)
    S^T   = K^T.T @ Q^T per (slot, key-tile), 1 PSUM bank, 4 rotating
    P^T   = exp(S^T/8 + bias) on ScalarE - valid_len mask is a
            per-partition bias column (0 / -30000)
    acc   = V1.T @ P^T accumulated over key tiles; rows 64:127 hold the
            denominator replicated across 64 partitions
    normalize per slot right after its last PV: reciprocal_approx_fast
    (DVE, ~5x faster than bit-exact) + one tensor_mul - no partition
    broadcast needed
    out_partial = heads^T.T @ Wo_slice, interleaved a few items into the
    NEXT chunk so its PSUM slots never starve the S-matmul pipeline
Host sums the 4 per-core partials of each batch element (the unshard for
the row-sharded Wo) and gathers.
"""
import sys
import numpy as np
from collections import deque
from contextlib import ExitStack

sys.path.insert(0, "/opt/trn_rl_repo")

import concourse.bass as bass  # noqa: E402
from concourse import bacc, mybir  # noqa: E402
import concourse.tile as tile  # noqa: E402
from concourse.bass_utils import run_bass_kernel_spmd  # noqa: E402

F32 = mybir.dt.float32
BF16 = mybir.dt.bfloat16
AF = mybir.ActivationFunctionType
NPBF16 = mybir.dt.np(BF16)

B, N, D, H = 2, 2048, 1024, 16
DH = 64
HPC = 4          # heads (slots) per core
NCORES = 8
QC = 512         # q chunk (matmul free dim)
NKT = N // 128   # 16 k tiles
NDC = D // 128   # 8 contraction chunks
MASK_BIAS = -30000.0
LOOKAHEAD = 3    # PV stagger (in items) behind S/exp emission

LAST_RESULTS = None  # BassKernelResults of the most recent run (for tooling)


def _build_program(trips):
    """trips: 4 ints (k-tile count per slot)."""
    nc = bacc.Bacc("TRN2", target_bir_lowering=False, debug=False,
                   num_devices=NCORES)

    xTq = nc.dram_tensor("xTq", [D, N], BF16, kind="ExternalInput")
    xTk = nc.dram_tensor("xTk", [D, N], BF16, kind="ExternalInput")
    xTv = nc.dram_tensor("xTv", [D, N], BF16, kind="ExternalInput")
    wq = nc.dram_tensor("wq", [128, NDC * 256], BF16, kind="ExternalInput")
    wk = nc.dram_tensor("wk", [128, NDC * 256], BF16, kind="ExternalInput")
    wv = nc.dram_tensor("wv", [128, NDC * 256], BF16, kind="ExternalInput")
    wo = nc.dram_tensor("wo", [256, D], BF16, kind="ExternalInput")
    vmask = nc.dram_tensor("vmask", [128, HPC * NKT], F32, kind="ExternalInput")
    out = nc.dram_tensor("out", [N, D], BF16, kind="ExternalOutput")

    # flat item list per chunk: round-robin tiles across slots so adjacent
    # items hit different slots (independent chains)
    items = []
    for r in range(max(trips)):
        for j in range(HPC):
            if r < trips[j]:
                items.append((j, r))

    with tile.TileContext(nc) as tc:
        with ExitStack() as ctx:
            wpool = ctx.enter_context(tc.tile_pool(name="wpool", bufs=1))
            xpool = ctx.enter_context(tc.tile_pool(name="xpool", bufs=6))
            qkpool = ctx.enter_context(tc.tile_pool(name="qkpool", bufs=1))
            v1pool = ctx.enter_context(tc.tile_pool(name="v1pool", bufs=1))
            ptpool = ctx.enter_context(tc.tile_pool(name="ptpool", bufs=8))
            nrmpool = ctx.enter_context(tc.tile_pool(name="nrmpool", bufs=4))
            pbpool = ctx.enter_context(tc.tile_pool(name="pbpool", bufs=1))
            opool = ctx.enter_context(tc.tile_pool(name="opool", bufs=8))

            t_wk = wpool.tile([128, NDC * 256], BF16, tag="wk")
            t_wq = wpool.tile([128, NDC * 256], BF16, tag="wq")
            t_wv = wpool.tile([128, NDC * 256], BF16, tag="wv")
            t_wo = [wpool.tile([128, D], BF16, tag=f"wo{p}", name=f"t_wo{p}")
                    for p in range(2)]
            t_vm = wpool.tile([128, HPC * NKT], F32, tag="vm")

            # K^T/Q^T: [128 dims (2 slots), N] per slot-pair
            t_kT = [qkpool.tile([128, N], BF16, tag=f"kT{p}", name=f"t_kT{p}")
                    for p in range(2)]
            t_qT = [qkpool.tile([128, N], BF16, tag=f"qT{p}", name=f"t_qT{p}")
                    for p in range(2)]
            # V1: per key-tile t, 4 blocks of [V_j (64 cols) | ones (64 cols)]
            t_v1 = v1pool.tile([128, NKT * HPC * 128], BF16, tag="v1")
            # normalized heads^T per slot pair: [128 dims, N]
            t_pb = [pbpool.tile([128, N], BF16, tag=f"pb{p}", name=f"t_pb{p}")
                    for p in range(2)]
            # scratch for the ACT exp-table preload
            t_pre = wpool.tile([1, 1], F32, tag="pre")

            # ones half-blocks of V1, one strided memset
            ones_ap = t_v1[:].rearrange(
                "p (b c) -> p b c", c=128)[:, :, 64:128]
            nc.vector.memset(ones_ap, 1.0)
            # preload the exp ACT table set while DMAs stream in
            nc.scalar.activation(t_pre[:], t_pre[:], AF.Exp)

            # ---- phase P: projections (K, Q, V) ----
            with tc.tile_pool(name="pp", bufs=8, space="PSUM") as pp:
                for si, (xin, wsb, dsts) in enumerate(
                        ((xTk, t_wk, t_kT), (xTq, t_wq, t_qT))):
                    accs = [pp.tile([128, QC], F32, tag="acc", name=f"acc_{i}")
                            for i in range(8)]
                    for c in range(NDC):
                        xt = xpool.tile([128, N], BF16, tag="xt")
                        nc.sync.dma_start(xt[:], xin[c * 128:(c + 1) * 128, :])
                        if si == 0:
                            # wk sliced per chunk: the first matmul only
                            # gates on 64KB of weights + one x chunk
                            nc.sync.dma_start(
                                t_wk[:, c * 256:(c + 1) * 256],
                                wk[:, c * 256:(c + 1) * 256])
                        for m in range(2):
                            for qq in range(4):
                                nc.tensor.matmul(
                                    accs[m * 4 + qq][:],
                                    wsb[:, c * 256 + m * 128:
                                        c * 256 + (m + 1) * 128],
                                    xt[:, qq * QC:(qq + 1) * QC],
                                    start=(c == 0), stop=(c == NDC - 1))
                    if si == 0:
                        nc.sync.dma_start(t_wq[:], wq[:])
                    else:
                        nc.sync.dma_start(t_wv[:], wv[:])
                    # drain PSUM->SBUF casts on BOTH ScalarE and DVE, q-chunk
                    # 0 first: the first attention S matmuls gate only on the
                    # chunk-0 casts, so phase A starts ~2.5us earlier
                    for n, i in enumerate((0, 4, 1, 5, 2, 6, 3, 7)):
                        dst = dsts[i // 4][:, (i % 4) * QC:(i % 4 + 1) * QC]
                        with nc.allow_low_precision(reason="f32r 4B"):
                            if n % 2 == 0:
                                nc.scalar.activation(dst, accs[i][:], AF.Copy)
                            else:
                                nc.vector.tensor_copy(dst, accs[i][:])
                nc.sync.dma_start(t_wo[0][:], wo[0:128, :])
                nc.sync.dma_start(t_wo[1][:], wo[128:256, :])
                nc.sync.dma_start(t_vm[:], vmask[:])
                # V projection: two half-column passes of 8 k-tiles
                for g in range(2):
                    accs = [pp.tile([128, 256], F32, tag="acc",
                                    name=f"accv_{i}") for i in range(8)]
                    for c in range(NDC):
                        xt = xpool.tile([128, 1024], BF16, tag="xtv")
                        nc.sync.dma_start(
                            xt[:], xTv[c * 128:(c + 1) * 128,
                                       g * 1024:(g + 1) * 1024])
                        for kt8 in range(8):
                            nc.tensor.matmul(
                                accs[kt8][:],
                                xt[:, kt8 * 128:(kt8 + 1) * 128],
                                t_wv[:, c * 256:(c + 1) * 256],
                                start=(c == 0), stop=(c == NDC - 1))
                    for kt8 in range(8):
                        t = g * 8 + kt8
                        # [128, 4, 64] strided copy: slot j -> V1 block
                        src = accs[kt8][:].rearrange("p (j c) -> p j c", c=64)
                        dst = t_v1[:, t * 512:(t + 1) * 512].rearrange(
                            "p (j c) -> p j c", c=128)[:, :, 0:64]
                        with nc.allow_low_precision(reason="f32r 4B"):
                            nc.vector.tensor_copy(dst, src)

            # ---- phase A: attention with fused output projection ----
            with tc.tile_pool(name="ap", bufs=1, space="PSUM") as ap:
                def emit_outproj_qt(qt):
                    ts = slice(qt * 128, (qt + 1) * 128)
                    stage = opool.tile([128, D], BF16, tag="ostage")
                    o_ps = [ap.tile([128, 512], F32, tag="sT", bufs=4,
                                    name=f"o_ps{ch}") for ch in range(2)]
                    for p2 in (1, 0):
                        for ch in range(2):
                            nc.tensor.matmul(
                                o_ps[ch][:], t_pb[p2][:, ts],
                                t_wo[p2][:, ch * 512:(ch + 1) * 512],
                                start=(p2 == 1), stop=(p2 == 0))
                    for ch in range(2):
                        with nc.allow_low_precision(reason="bf16 out"):
                            nc.vector.tensor_copy(
                                stage[:, ch * 512:(ch + 1) * 512],
                                o_ps[ch][:])
                    nc.sync.dma_start(out[ts, :], stage[:])

                nitems = len(items)
                last_of_slot = {}
                for i, (j, t) in enumerate(items):
                    last_of_slot[j] = i

                for q in range(N // QC):
                    qs = slice(q * QC, (q + 1) * QC)
                    accs2 = [ap.tile([128, QC], F32, tag="acc2", bufs=4,
                                     name=f"acc_{j}") for j in range(HPC)]
                    pend = deque()
                    seen = [0] * HPC

                    def emit_pv(j, t, pt):
                        base = (t * HPC + j) * 128
                        seen[j] += 1
                        nc.tensor.matmul(
                            accs2[j][:], t_v1[:, base:base + 128], pt[:],
                            start=(seen[j] == 1), stop=(seen[j] == trips[j]))
                        if seen[j] == trips[j]:
                            # normalize right after the slot's last PV:
                            # denominator is pre-broadcast in rows 64:127
                            p, half = j // 2, j % 2
                            rows = slice(half * 64, (half + 1) * 64)
                            den = nrmpool.tile([64, QC], F32, tag="den")
                            nc.vector.tensor_copy(
                                den[:], accs2[j][64:128, :])
                            rcp = nrmpool.tile([64, QC], F32, tag="rcp")
                            nc.vector.reciprocal_approx_fast(rcp[:], den[:])
                            with nc.allow_low_precision(reason="f32r 4B"):
                                nc.vector.tensor_mul(
                                    t_pb[p][rows, qs], accs2[j][0:64, :],
                                    rcp[:])

                    # outproj of the previous chunk, spread through this
                    # chunk's item stream (one 128-query tile at a time)
                    op_sched = {}
                    if q > 0:
                        step = max(1, (nitems - 4) // 4)
                        for k in range(4):
                            op_sched[min(2 + k * step, nitems - 1)] = k

                    for i, (j, t) in enumerate(items):
                        p, half = j // 2, j % 2
                        rows = slice(half * 64, (half + 1) * 64)
                        sT = ap.tile([128, QC], F32, tag="sT", bufs=4)
                        nc.tensor.matmul(
                            sT[:], t_kT[p][rows, t * 128:(t + 1) * 128],
                            t_qT[p][rows, qs], start=True, stop=True)
                        pT = ptpool.tile([128, QC], BF16, tag="pT")
                        nc.scalar.activation(
                            pT[:], sT[:], AF.Exp, scale=0.125,
                            bias=t_vm[:, j * NKT + t: j * NKT + t + 1])
                        pend.append((j, t, pT))
                        if len(pend) > LOOKAHEAD:
                            emit_pv(*pend.popleft())
                        if i in op_sched:
                            emit_outproj_qt((q - 1) * 4 + op_sched[i])
                    while pend:
                        emit_pv(*pend.popleft())
                # last chunk's output projection
                for qt in range((N // QC - 1) * 4, (N // QC) * 4):
                    emit_outproj_qt(qt)

    nc.finalize()
    return nc


def kernel(queries, keys, values, valid_len, Wq, Wk, Wv, Wo):
    global LAST_RESULTS
    queries = np.asarray(queries, dtype=np.float32)
    keys = np.asarray(keys, dtype=np.float32)
    values = np.asarray(values, dtype=np.float32)
    Wq = np.asarray(Wq, dtype=np.float32)
    Wk = np.asarray(Wk, dtype=np.float32)
    Wv = np.asarray(Wv, dtype=np.float32)
    Wo = np.asarray(Wo, dtype=np.float32)
    vl = np.asarray(valid_len).astype(np.int64).reshape(B * H)

    # rank-aligned slot assignment: per batch, heads sorted by vl desc;
    # slot j of the 4 cores of that batch takes ranks 4j..4j+3
    order = {}
    for b in range(B):
        idx = (np.argsort(-vl[b * H:(b + 1) * H], kind="stable") + b * H)
        for cg in range(4):
            order[b * 4 + cg] = [int(idx[4 * j + cg]) for j in range(HPC)]
    trips = []
    for j in range(HPC):
        vs = [int(vl[order[c][j]]) for c in range(NCORES)]
        m = max(-(-v // 128) for v in vs)
        trips.append(max(1, min(NKT, m)))

    nc = _build_program(tuple(trips))

    in_maps = []
    for c in range(NCORES):
        b = c // 4
        heads = order[c]
        cols = np.concatenate(
            [np.arange((h - b * H) * DH, (h - b * H + 1) * DH) for h in heads])

        def wlayout(w):
            return np.ascontiguousarray(
                w[:, cols].reshape(NDC, 128, 256).transpose(1, 0, 2)
                .reshape(128, NDC * 256).astype(NPBF16))

        vm = np.zeros((128, HPC * NKT), np.float32)
        for j, h in enumerate(heads):
            bias = np.where(np.arange(N) < vl[h], 0.0, MASK_BIAS)
            vm[:, j * NKT:(j + 1) * NKT] = bias.reshape(NKT, 128).T

        in_maps.append({
            "xTq": np.ascontiguousarray(queries[b].T.astype(NPBF16)),
            "xTk": np.ascontiguousarray(keys[b].T.astype(NPBF16)),
            "xTv": np.ascontiguousarray(values[b].T.astype(NPBF16)),
            "wq": wlayout(Wq),
            "wk": wlayout(Wk),
            "wv": wlayout(Wv),
            "wo": np.ascontiguousarray(Wo[cols, :]).astype(NPBF16),
            "vmask": vm,
        })

    LAST_RESULTS = run_bass_kernel_spmd(nc, in_maps, list(range(NCORES)))
    res = LAST_RESULTS.results

    out = np.zeros((B, N, D), np.float64)
    for c in range(NCORES):
        out[c // 4] += res[c]["out"].astype(np.float64)
    return out.astype(np.float32)


# revision 13
# speedup vs baseline: 1.4205x; 1.1575x over previous
"""Multi-head attention TRN2 kernel (8 NeuronCores, SPMD).

Problem: B=2, N=2048, D=1024, H=16 heads of dim 64, fp32, per-(b,h)
key-length masking (valid_len, length 32).

Sharding: batch*heads across 8 cores - core c handles batch b=c//4 and 4
heads ("slots", rank-aligned by valid_len so the SPMD trip counts stay
balanced).  Per core:

  phase P (projections, bf16 inputs to halve HBM traffic):
    K^T/Q^T = Wslice^T @ x^T   (head dims on partitions, positions free)
    V       = x^T-tiles as lhsT, Wv as rhs  (positions on partitions),
              copied into V1 = [V_j | ones(64)] blocks per (key-tile, slot)
              - the 64 replicated ones columns make the PV matmul emit the
              softmax denominator PRE-BROADCAST in PSUM rows 64:127
  phase A (attention, all-bf16 operands, f32 PSUM accumulate):
    flat round-robin over (slot, key-tile) items per 512-query chunk with
    a PV lookahead stagger: S(i) and exp(i) are emitted immediately, but
    PV(i-L) is emitted L items later, so a PV waiting on its exp never
    head-of-line-blocks the next S matmul in the PE's strict-FIFO queue
    (the previous interleave serialized S->exp->PV per group, leaving
    both PE and ScalarE ~50% idle and oscillating the HAM clock gate)
    S^T   = K^T.T @ Q^T per (slot, key-tile), 1 PSUM bank, 4 rotating
    P^T   = exp(S^T/8 + bias) on ScalarE - valid_len mask is a
            per-partition bias column (0 / -30000)
    acc   = V1.T @ P^T accumulated over key tiles; rows 64:127 hold the
            denominator replicated across 64 partitions
    normalize per slot right after its last PV: reciprocal_approx_fast
    (DVE, ~5x faster than bit-exact) + one tensor_mul - no partition
    broadcast needed
    out_partial = heads^T.T @ Wo_slice, interleaved a few items into the
    NEXT chunk so its PSUM slots never starve the S-matmul pipeline
Host sums the 4 per-core partials of each batch element (the unshard for
the row-sharded Wo) and gathers.
"""
import sys
import numpy as np
from collections import deque
from contextlib import ExitStack

sys.path.insert(0, "/opt/trn_rl_repo")

import concourse.bass as bass  # noqa: E402
from concourse import bacc, mybir  # noqa: E402
import concourse.tile as tile  # noqa: E402
from concourse.bass_utils import run_bass_kernel_spmd  # noqa: E402

F32 = mybir.dt.float32
BF16 = mybir.dt.bfloat16
AF = mybir.ActivationFunctionType
NPBF16 = mybir.dt.np(BF16)

B, N, D, H = 2, 2048, 1024, 16
DH = 64
HPC = 4          # heads (slots) per core
NCORES = 8
QC = 512         # q chunk (matmul free dim)
NKT = N // 128   # 16 k tiles
NDC = D // 128   # 8 contraction chunks
MASK_BIAS = -30000.0
LOOKAHEAD = 3    # PV stagger (in items) behind S/exp emission

LAST_RESULTS = None  # BassKernelResults of the most recent run (for tooling)


def _build_program(trips):
    """trips: 4 ints (k-tile count per slot)."""
    nc = bacc.Bacc("TRN2", target_bir_lowering=False, debug=False,
                   num_devices=NCORES)

    xTq = nc.dram_tensor("xTq", [D, N], BF16, kind="ExternalInput")
    xTk = nc.dram_tensor("xTk", [D, N], BF16, kind="ExternalInput")
    xTv = nc.dram_tensor("xTv", [D, N], BF16, kind="ExternalInput")
    wq = nc.dram_tensor("wq", [128, NDC * 256], BF16, kind="ExternalInput")
    wk = nc.dram_tensor("wk", [128, NDC * 256], BF16, kind="ExternalInput")
    wv = nc.dram_tensor("wv", [128, NDC * 256], BF16, kind="ExternalInput")
    wo = nc.dram_tensor("wo", [256, D], BF16, kind="ExternalInput")
    vmask = nc.dram_tensor("vmask", [128, HPC * NKT], F32, kind="ExternalInput")
    out = nc.dram_tensor("out", [N, D], BF16, kind="ExternalOutput")

    # flat item list per chunk: round-robin tiles across slots so adjacent
    # items hit different slots (independent chains)
    items = []
    for r in range(max(trips)):
        for j in range(HPC):
            if r < trips[j]:
                items.append((j, r))

    with tile.TileContext(nc) as tc:
        with ExitStack() as ctx:
            wpool = ctx.enter_context(tc.tile_pool(name="wpool", bufs=1))
            xpool = ctx.enter_context(tc.tile_pool(name="xpool", bufs=6))
            qkpool = ctx.enter_context(tc.tile_pool(name="qkpool", bufs=1))
            v1pool = ctx.enter_context(tc.tile_pool(name="v1pool", bufs=1))
            ptpool = ctx.enter_context(tc.tile_pool(name="ptpool", bufs=40))
            nrmpool = ctx.enter_context(tc.tile_pool(name="nrmpool", bufs=4))
            pbpool = ctx.enter_context(tc.tile_pool(name="pbpool", bufs=1))
            opool = ctx.enter_context(tc.tile_pool(name="opool", bufs=8))

            t_wk = wpool.tile([128, NDC * 256], BF16, tag="wk")
            t_wq = wpool.tile([128, NDC * 256], BF16, tag="wq")
            t_wv = wpool.tile([128, NDC * 256], BF16, tag="wv")
            t_wo = [wpool.tile([128, D], BF16, tag=f"wo{p}", name=f"t_wo{p}")
                    for p in range(2)]
            t_vm = wpool.tile([128, HPC * NKT], F32, tag="vm")

            # K^T/Q^T: [128 dims (2 slots), N] per slot-pair
            t_kT = [qkpool.tile([128, N], BF16, tag=f"kT{p}", name=f"t_kT{p}")
                    for p in range(2)]
            t_qT = [qkpool.tile([128, N], BF16, tag=f"qT{p}", name=f"t_qT{p}")
                    for p in range(2)]
            # V1: per key-tile t, 4 blocks of [V_j (64 cols) | ones (64 cols)]
            t_v1 = v1pool.tile([128, NKT * HPC * 128], BF16, tag="v1")
            # normalized heads^T per slot pair: [128 dims, N]
            t_pb = [pbpool.tile([128, N], BF16, tag=f"pb{p}", name=f"t_pb{p}")
                    for p in range(2)]
            # scratch for the ACT exp-table preload
            t_pre = wpool.tile([1, 1], F32, tag="pre")

            # ones half-blocks of V1, one strided memset
            ones_ap = t_v1[:].rearrange(
                "p (b c) -> p b c", c=128)[:, :, 64:128]
            nc.vector.memset(ones_ap, 1.0)
            # preload the exp ACT table set while DMAs stream in
            nc.scalar.activation(t_pre[:], t_pre[:], AF.Exp)

            # ---- phase P: projections (K, Q, V) ----
            with tc.tile_pool(name="pp", bufs=8, space="PSUM") as pp:
                for si, (xin, wsb, dsts) in enumerate(
                        ((xTk, t_wk, t_kT), (xTq, t_wq, t_qT))):
                    accs = [pp.tile([128, QC], F32, tag="acc", name=f"acc_{i}")
                            for i in range(8)]
                    for c in range(NDC):
                        xt = xpool.tile([128, N], BF16, tag="xt")
                        if si == 0 and c == 0:
                            # split the gating first chunk across 4 DMA
                            # queues so the first matmul fires sooner
                            for sp in range(4):
                                cs = slice(sp * 512, (sp + 1) * 512)
                                nc.sync.dma_start(
                                    xt[:, cs], xin[0:128, cs])
                        else:
                            nc.sync.dma_start(
                                xt[:], xin[c * 128:(c + 1) * 128, :])
                        if si == 0:
                            # wk sliced per chunk: the first matmul only
                            # gates on 64KB of weights + one x chunk
                            nc.sync.dma_start(
                                t_wk[:, c * 256:(c + 1) * 256],
                                wk[:, c * 256:(c + 1) * 256])
                        for m in range(2):
                            for qq in range(4):
                                nc.tensor.matmul(
                                    accs[m * 4 + qq][:],
                                    wsb[:, c * 256 + m * 128:
                                        c * 256 + (m + 1) * 128],
                                    xt[:, qq * QC:(qq + 1) * QC],
                                    start=(c == 0), stop=(c == NDC - 1))
                    if si == 0:
                        nc.sync.dma_start(t_wq[:], wq[:])
                    else:
                        nc.sync.dma_start(t_wv[:], wv[:])
                    # drain PSUM->SBUF casts on BOTH ScalarE and DVE, q-chunk
                    # 0 first: the first attention S matmuls gate only on the
                    # chunk-0 casts, so phase A starts ~2.5us earlier
                    for n, i in enumerate((0, 4, 1, 5, 2, 6, 3, 7)):
                        dst = dsts[i // 4][:, (i % 4) * QC:(i % 4 + 1) * QC]
                        with nc.allow_low_precision(reason="f32r 4B"):
                            if n % 2 == 0:
                                nc.scalar.activation(dst, accs[i][:], AF.Copy)
                            else:
                                nc.vector.tensor_copy(dst, accs[i][:])
                nc.sync.dma_start(t_wo[0][:], wo[0:128, :])
                nc.sync.dma_start(t_wo[1][:], wo[128:256, :])
                nc.sync.dma_start(t_vm[:], vmask[:])

            # ---- phase A: attention with V projection streamed into chunk
            # 0 (fills PE bubbles while ScalarE chews chunk-0 exps) and
            # fused output projection ----
            with tc.tile_pool(name="ap", bufs=1, space="PSUM") as ap:
                # V projection, split into 16 c-steps interleaved with the
                # chunk-0 S/exp stream.  PSUM: the two 4-bank pass groups
                # come from the same rotation ("acc2" tag) that the
                # attention accumulators use later - acc2 tiles allocate
                # only after V is done, so 4 sT + 4 V banks = 8 exactly.
                vaccs = [None]

                def v_step(sp, c):
                    # sub-pass sp covers 4 key-tiles (bank-aligned [128,512]
                    # slots, V data in cols 0:256), streamed over 8 c-steps
                    g, h = sp // 2, sp % 2
                    if c == 0:
                        vaccs[0] = [ap.tile([128, 512], F32, tag="acc2",
                                            bufs=4, name=f"vacc{sp}_{i}")
                                    for i in range(4)]
                    xt = xpool.tile([128, 512], BF16, tag="xtv")
                    nc.sync.dma_start(
                        xt[:], xTv[c * 128:(c + 1) * 128,
                                   g * 1024 + h * 512:
                                   g * 1024 + (h + 1) * 512])
                    for k in range(4):
                        nc.tensor.matmul(
                            vaccs[0][k][:, 0:256],
                            xt[:, k * 128:(k + 1) * 128],
                            t_wv[:, c * 256:(c + 1) * 256],
                            start=(c == 0), stop=(c == NDC - 1))
                    if c == NDC - 1:
                        for k in range(4):
                            t = g * 8 + h * 4 + k
                            # [128, 4, 64] strided copy: slot j -> V1 block
                            src = vaccs[0][k][:, 0:256].rearrange(
                                "p (j c) -> p j c", c=64)
                            dst = t_v1[:, t * 512:(t + 1) * 512].rearrange(
                                "p (j c) -> p j c", c=128)[:, :, 0:64]
                            with nc.allow_low_precision(reason="f32r 4B"):
                                nc.vector.tensor_copy(dst, src)
                def emit_outproj_qt(qt):
                    ts = slice(qt * 128, (qt + 1) * 128)
                    stage = opool.tile([128, D], BF16, tag="ostage")
                    o_ps = [ap.tile([128, 512], F32, tag="sT", bufs=4,
                                    name=f"o_ps{ch}") for ch in range(2)]
                    for p2 in (1, 0):
                        for ch in range(2):
                            nc.tensor.matmul(
                                o_ps[ch][:], t_pb[p2][:, ts],
                                t_wo[p2][:, ch * 512:(ch + 1) * 512],
                                start=(p2 == 1), stop=(p2 == 0))
                    for ch in range(2):
                        with nc.allow_low_precision(reason="bf16 out"):
                            nc.vector.tensor_copy(
                                stage[:, ch * 512:(ch + 1) * 512],
                                o_ps[ch][:])
                    nc.sync.dma_start(out[ts, :], stage[:])

                nitems = len(items)
                for q in range(N // QC):
                    qs = slice(q * QC, (q + 1) * QC)
                    accs2 = []  # allocated lazily, after V frees its banks
                    pend = deque()
                    seen = [0] * HPC

                    def ensure_accs2():
                        if not accs2:
                            accs2.extend(
                                ap.tile([128, QC], F32, tag="acc2", bufs=4,
                                        name=f"acc_{j}")
                                for j in range(HPC))

                    def emit_pv(j, t, pt):
                        base = (t * HPC + j) * 128
                        seen[j] += 1
                        nc.tensor.matmul(
                            accs2[j][:], t_v1[:, base:base + 128], pt[:],
                            start=(seen[j] == 1), stop=(seen[j] == trips[j]))
                        if seen[j] == trips[j]:
                            # normalize right after the slot's last PV:
                            # denominator is pre-broadcast in rows 64:127
                            p, half = j // 2, j % 2
                            rows = slice(half * 64, (half + 1) * 64)
                            den = nrmpool.tile([64, QC], F32, tag="den")
                            nc.vector.tensor_copy(
                                den[:], accs2[j][64:128, :])
                            rcp = nrmpool.tile([64, QC], F32, tag="rcp")
                            nc.vector.reciprocal_approx_fast(rcp[:], den[:])
                            with nc.allow_low_precision(reason="f32r 4B"):
                                nc.vector.tensor_mul(
                                    t_pb[p][rows, qs], accs2[j][0:64, :],
                                    rcp[:])

                    # chunk 0 carries the 16 V-projection c-steps spread
                    # over its first ~2/3 items (PVs defer until V's PSUM
                    # banks are free); later chunks carry the previous
                    # chunk's output projection instead
                    vsched, op_sched = {}, {}
                    if q == 0:
                        span = max(32, int(nitems * 0.75))
                        for s in range(32):
                            vsched.setdefault(
                                min(1 + s * span // 32, nitems - 1), []
                            ).append(s)
                        vleft = 32
                    else:
                        vleft = 0
                        step = max(1, (nitems - 4) // 4)
                        for k in range(4):
                            op_sched[min(2 + k * step, nitems - 1)] = k

                    for i, (j, t) in enumerate(items):
                        p, half = j // 2, j % 2
                        rows = slice(half * 64, (half + 1) * 64)
                        sT = ap.tile([128, QC], F32, tag="sT", bufs=4)
                        nc.tensor.matmul(
                            sT[:], t_kT[p][rows, t * 128:(t + 1) * 128],
                            t_qT[p][rows, qs], start=True, stop=True)
                        pT = ptpool.tile([128, QC], BF16, tag="pT")
                        nc.scalar.activation(
                            pT[:], sT[:], AF.Exp, scale=0.125,
                            bias=t_vm[:, j * NKT + t: j * NKT + t + 1])
                        pend.append((j, t, pT))
                        for s in vsched.get(i, ()):
                            v_step(s // 8, s % 8)
                            vleft -= 1
                        if vleft == 0:
                            ensure_accs2()
                            budget = 2 if q == 0 else 1
                            while len(pend) > LOOKAHEAD and budget:
                                emit_pv(*pend.popleft())
                                budget -= 1
                        if i in op_sched:
                            emit_outproj_qt((q - 1) * 4 + op_sched[i])
                    ensure_accs2()
                    while pend:
                        emit_pv(*pend.popleft())
                # last chunk's output projection
                for qt in range((N // QC - 1) * 4, (N // QC) * 4):
                    emit_outproj_qt(qt)

    nc.finalize()
    return nc


def kernel(queries, keys, values, valid_len, Wq, Wk, Wv, Wo):
    global LAST_RESULTS
    queries = np.asarray(queries, dtype=np.float32)
    keys = np.asarray(keys, dtype=np.float32)
    values = np.asarray(values, dtype=np.float32)
    Wq = np.asarray(Wq, dtype=np.float32)
    Wk = np.asarray(Wk, dtype=np.float32)
    Wv = np.asarray(Wv, dtype=np.float32)
    Wo = np.asarray(Wo, dtype=np.float32)
    vl = np.asarray(valid_len).astype(np.int64).reshape(B * H)

    # rank-aligned slot assignment: per batch, heads sorted by vl desc;
    # slot j of the 4 cores of that batch takes ranks 4j..4j+3
    order = {}
    for b in range(B):
        idx = (np.argsort(-vl[b * H:(b + 1) * H], kind="stable") + b * H)
        for cg in range(4):
            order[b * 4 + cg] = [int(idx[4 * j + cg]) for j in range(HPC)]
    trips = []
    for j in range(HPC):
        vs = [int(vl[order[c][j]]) for c in range(NCORES)]
        m = max(-(-v // 128) for v in vs)
        trips.append(max(1, min(NKT, m)))

    nc = _build_program(tuple(trips))

    in_maps = []
    for c in range(NCORES):
        b = c // 4
        heads = order[c]
        cols = np.concatenate(
            [np.arange((h - b * H) * DH, (h - b * H + 1) * DH) for h in heads])

        def wlayout(w):
            return np.ascontiguousarray(
                w[:, cols].reshape(NDC, 128, 256).transpose(1, 0, 2)
                .reshape(128, NDC * 256).astype(NPBF16))

        vm = np.zeros((128, HPC * NKT), np.float32)
        for j, h in enumerate(heads):
            bias = np.where(np.arange(N) < vl[h], 0.0, MASK_BIAS)
            vm[:, j * NKT:(j + 1) * NKT] = bias.reshape(NKT, 128).T

        in_maps.append({
            "xTq": np.ascontiguousarray(queries[b].T.astype(NPBF16)),
            "xTk": np.ascontiguousarray(keys[b].T.astype(NPBF16)),
            "xTv": np.ascontiguousarray(values[b].T.astype(NPBF16)),
            "wq": wlayout(Wq),
            "wk": wlayout(Wk),
            "wv": wlayout(Wv),
            "wo": np.ascontiguousarray(Wo[cols, :]).astype(NPBF16),
            "vmask": vm,
        })

    LAST_RESULTS = run_bass_kernel_spmd(nc, in_maps, list(range(NCORES)))
    res = LAST_RESULTS.results

    out = np.zeros((B, N, D), np.float64)
    for c in range(NCORES):
        out[c // 4] += res[c]["out"].astype(np.float64)
    return out.astype(np.float32)


# revision 14
# speedup vs baseline: 1.4777x; 1.0403x over previous
"""Multi-head attention TRN2 kernel (8 NeuronCores, SPMD).

Problem: B=2, N=2048, D=1024, H=16 heads of dim 64, fp32, per-(b,h)
key-length masking (valid_len, length 32).

Sharding: batch*heads across 8 cores - core c handles batch b=c//4 and 4
heads ("slots", rank-aligned by valid_len so the SPMD trip counts stay
balanced).  Per core:

  phase P (projections, bf16 inputs to halve HBM traffic):
    K^T/Q^T = Wslice^T @ x^T   (head dims on partitions, positions free)
    V       = x^T-tiles as lhsT, Wv as rhs  (positions on partitions),
              copied into V1 = [V_j | ones(64)] blocks per (key-tile, slot)
              - the 64 replicated ones columns make the PV matmul emit the
              softmax denominator PRE-BROADCAST in PSUM rows 64:127
  phase A (attention, all-bf16 operands, f32 PSUM accumulate):
    flat round-robin over (slot, key-tile) items per 512-query chunk with
    a PV lookahead stagger: S(i) and exp(i) are emitted immediately, but
    PV(i-L) is emitted L items later, so a PV waiting on its exp never
    head-of-line-blocks the next S matmul in the PE's strict-FIFO queue
    (the previous interleave serialized S->exp->PV per group, leaving
    both PE and ScalarE ~50% idle and oscillating the HAM clock gate)
    S^T   = K^T.T @ Q^T per (slot, key-tile), 1 PSUM bank, 4 rotating
    P^T   = exp(S^T/8 + bias) on ScalarE - valid_len mask is a
            per-partition bias column (0 / -30000)
    acc   = V1.T @ P^T accumulated over key tiles; rows 64:127 hold the
            denominator replicated across 64 partitions
    normalize per slot right after its last PV: reciprocal_approx_fast
    (DVE, ~5x faster than bit-exact) + one tensor_mul - no partition
    broadcast needed
    out_partial = heads^T.T @ Wo_slice, interleaved a few items into the
    NEXT chunk so its PSUM slots never starve the S-matmul pipeline
Host sums the 4 per-core partials of each batch element (the unshard for
the row-sharded Wo) and gathers.
"""
import sys
import numpy as np
from collections import deque
from contextlib import ExitStack

sys.path.insert(0, "/opt/trn_rl_repo")

import concourse.bass as bass  # noqa: E402
from concourse import bacc, mybir  # noqa: E402
import concourse.tile as tile  # noqa: E402
from concourse.bass_utils import run_bass_kernel_spmd  # noqa: E402

F32 = mybir.dt.float32
BF16 = mybir.dt.bfloat16
AF = mybir.ActivationFunctionType
NPBF16 = mybir.dt.np(BF16)

B, N, D, H = 2, 2048, 1024, 16
DH = 64
HPC = 4          # heads (slots) per core
NCORES = 8
QC = 512         # q chunk (matmul free dim)
NKT = N // 128   # 16 k tiles
NDC = D // 128   # 8 contraction chunks
MASK_BIAS = -30000.0
LOOKAHEAD = 3    # PV stagger (in items) behind S/exp emission

LAST_RESULTS = None  # BassKernelResults of the most recent run (for tooling)


def _build_program(trips):
    """trips: 4 ints (k-tile count per slot)."""
    nc = bacc.Bacc("TRN2", target_bir_lowering=False, debug=False,
                   num_devices=NCORES)

    xTq = nc.dram_tensor("xTq", [D, N], BF16, kind="ExternalInput")
    xTk = nc.dram_tensor("xTk", [D, N], BF16, kind="ExternalInput")
    xTv = nc.dram_tensor("xTv", [D, N], BF16, kind="ExternalInput")
    wq = nc.dram_tensor("wq", [NDC * 128, 256], BF16, kind="ExternalInput")
    wk = nc.dram_tensor("wk", [NDC * 128, 256], BF16, kind="ExternalInput")
    wv = nc.dram_tensor("wv", [NDC * 128, 256], BF16, kind="ExternalInput")
    wo = nc.dram_tensor("wo", [256, D], BF16, kind="ExternalInput")
    vmask = nc.dram_tensor("vmask", [128, HPC * NKT], F32, kind="ExternalInput")
    out = nc.dram_tensor("out", [N, D], BF16, kind="ExternalOutput")

    # flat item list per chunk: round-robin tiles across slots so adjacent
    # items hit different slots (independent chains)
    items = []
    for r in range(max(trips)):
        for j in range(HPC):
            if r < trips[j]:
                items.append((j, r))

    with tile.TileContext(nc) as tc:
        with ExitStack() as ctx:
            wpool = ctx.enter_context(tc.tile_pool(name="wpool", bufs=1))
            xpool = ctx.enter_context(tc.tile_pool(name="xpool", bufs=6))
            qkpool = ctx.enter_context(tc.tile_pool(name="qkpool", bufs=1))
            v1pool = ctx.enter_context(tc.tile_pool(name="v1pool", bufs=1))
            ptpool = ctx.enter_context(tc.tile_pool(name="ptpool", bufs=40))
            nrmpool = ctx.enter_context(tc.tile_pool(name="nrmpool", bufs=4))
            pbpool = ctx.enter_context(tc.tile_pool(name="pbpool", bufs=1))
            opool = ctx.enter_context(tc.tile_pool(name="opool", bufs=8))

            t_wk = wpool.tile([128, NDC * 256], BF16, tag="wk")
            t_wq = wpool.tile([128, NDC * 256], BF16, tag="wq")
            t_wv = wpool.tile([128, NDC * 256], BF16, tag="wv")
            t_wo = [wpool.tile([128, D], BF16, tag=f"wo{p}", name=f"t_wo{p}")
                    for p in range(2)]
            t_vm = wpool.tile([128, HPC * NKT], F32, tag="vm")

            # K^T/Q^T: [128 dims (2 slots), N] per slot-pair
            t_kT = [qkpool.tile([128, N], BF16, tag=f"kT{p}", name=f"t_kT{p}")
                    for p in range(2)]
            t_qT = [qkpool.tile([128, N], BF16, tag=f"qT{p}", name=f"t_qT{p}")
                    for p in range(2)]
            # V1: per key-tile t, 4 blocks of [V_j (64 cols) | ones (64 cols)]
            t_v1 = v1pool.tile([128, NKT * HPC * 128], BF16, tag="v1")
            # normalized heads^T per slot pair: [128 dims, N]
            t_pb = [pbpool.tile([128, N], BF16, tag=f"pb{p}", name=f"t_pb{p}")
                    for p in range(2)]
            # scratch for the ACT exp-table preload
            t_pre = wpool.tile([1, 1], F32, tag="pre")

            # ones half-blocks of V1, one strided memset
            ones_ap = t_v1[:].rearrange(
                "p (b c) -> p b c", c=128)[:, :, 64:128]
            nc.vector.memset(ones_ap, 1.0)
            # preload the exp ACT table set while DMAs stream in
            nc.scalar.activation(t_pre[:], t_pre[:], AF.Exp)

            # ---- phase P: projections (K, Q, V) ----
            with tc.tile_pool(name="pp", bufs=8, space="PSUM") as pp:
                for si, (xin, wsb, dsts) in enumerate(
                        ((xTk, t_wk, t_kT), (xTq, t_wq, t_qT))):
                    accs = [pp.tile([128, QC], F32, tag="acc", name=f"acc_{i}")
                            for i in range(8)]
                    for c in range(NDC):
                        xt = xpool.tile([128, N], BF16, tag="xt")
                        if si == 0 and c == 0:
                            # split the gating first chunk across 4 DMA
                            # queues so the first matmul fires sooner
                            for sp in range(4):
                                cs = slice(sp * 512, (sp + 1) * 512)
                                nc.sync.dma_start(
                                    xt[:, cs], xin[0:128, cs])
                        else:
                            nc.sync.dma_start(
                                xt[:], xin[c * 128:(c + 1) * 128, :])
                        if si == 0:
                            # wk sliced per chunk: the first matmul only
                            # gates on 64KB of weights + one x chunk
                            nc.sync.dma_start(
                                t_wk[:, c * 256:(c + 1) * 256],
                                wk[c * 128:(c + 1) * 128, :])
                        for m in range(2):
                            for qq in range(4):
                                nc.tensor.matmul(
                                    accs[m * 4 + qq][:],
                                    wsb[:, c * 256 + m * 128:
                                        c * 256 + (m + 1) * 128],
                                    xt[:, qq * QC:(qq + 1) * QC],
                                    start=(c == 0), stop=(c == NDC - 1))
                    wnext, tnext = (wq, t_wq) if si == 0 else (wv, t_wv)
                    for c in range(NDC):
                        nc.sync.dma_start(
                            tnext[:, c * 256:(c + 1) * 256],
                            wnext[c * 128:(c + 1) * 128, :])
                    # drain PSUM->SBUF casts on BOTH ScalarE and DVE, q-chunk
                    # 0 first: the first attention S matmuls gate only on the
                    # chunk-0 casts, so phase A starts ~2.5us earlier
                    for n, i in enumerate((0, 4, 1, 5, 2, 6, 3, 7)):
                        dst = dsts[i // 4][:, (i % 4) * QC:(i % 4 + 1) * QC]
                        with nc.allow_low_precision(reason="f32r 4B"):
                            if n % 2 == 0:
                                nc.scalar.activation(dst, accs[i][:], AF.Copy)
                            else:
                                nc.vector.tensor_copy(dst, accs[i][:])
                nc.sync.dma_start(t_wo[0][:], wo[0:128, :])
                nc.sync.dma_start(t_wo[1][:], wo[128:256, :])
                nc.sync.dma_start(t_vm[:], vmask[:])

            # ---- phase A: attention with V projection streamed into chunk
            # 0 (fills PE bubbles while ScalarE chews chunk-0 exps) and
            # fused output projection ----
            with tc.tile_pool(name="ap", bufs=1, space="PSUM") as ap:
                # V projection, split into 16 c-steps interleaved with the
                # chunk-0 S/exp stream.  PSUM: the two 4-bank pass groups
                # come from the same rotation ("acc2" tag) that the
                # attention accumulators use later - acc2 tiles allocate
                # only after V is done, so 4 sT + 4 V banks = 8 exactly.
                vaccs = [None]

                def v_step(sp, c):
                    # sub-pass sp covers 4 key-tiles (bank-aligned [128,512]
                    # slots, V data in cols 0:256), streamed over 8 c-steps
                    g, h = sp // 2, sp % 2
                    if c == 0:
                        vaccs[0] = [ap.tile([128, 512], F32, tag="acc2",
                                            bufs=4, name=f"vacc{sp}_{i}")
                                    for i in range(4)]
                    xt = xpool.tile([128, 512], BF16, tag="xtv")
                    nc.sync.dma_start(
                        xt[:], xTv[c * 128:(c + 1) * 128,
                                   g * 1024 + h * 512:
                                   g * 1024 + (h + 1) * 512])
                    for k in range(4):
                        nc.tensor.matmul(
                            vaccs[0][k][:, 0:256],
                            xt[:, k * 128:(k + 1) * 128],
                            t_wv[:, c * 256:(c + 1) * 256],
                            start=(c == 0), stop=(c == NDC - 1))
                    if c == NDC - 1:
                        for k in range(4):
                            t = g * 8 + h * 4 + k
                            # [128, 4, 64] strided copy: slot j -> V1 block
                            src = vaccs[0][k][:, 0:256].rearrange(
                                "p (j c) -> p j c", c=64)
                            dst = t_v1[:, t * 512:(t + 1) * 512].rearrange(
                                "p (j c) -> p j c", c=128)[:, :, 0:64]
                            with nc.allow_low_precision(reason="f32r 4B"):
                                nc.vector.tensor_copy(dst, src)
                def emit_outproj_qt(qt):
                    ts = slice(qt * 128, (qt + 1) * 128)
                    stage = opool.tile([128, D], BF16, tag="ostage")
                    o_ps = [ap.tile([128, 512], F32, tag="sT", bufs=4,
                                    name=f"o_ps{ch}") for ch in range(2)]
                    for p2 in (1, 0):
                        for ch in range(2):
                            nc.tensor.matmul(
                                o_ps[ch][:], t_pb[p2][:, ts],
                                t_wo[p2][:, ch * 512:(ch + 1) * 512],
                                start=(p2 == 1), stop=(p2 == 0))
                    for ch in range(2):
                        with nc.allow_low_precision(reason="bf16 out"):
                            nc.vector.tensor_copy(
                                stage[:, ch * 512:(ch + 1) * 512],
                                o_ps[ch][:])
                    nc.sync.dma_start(out[ts, :], stage[:])

                nitems = len(items)
                for q in range(N // QC):
                    qs = slice(q * QC, (q + 1) * QC)
                    accs2 = []  # allocated lazily, after V frees its banks
                    pend = deque()
                    seen = [0] * HPC

                    def ensure_accs2():
                        if not accs2:
                            accs2.extend(
                                ap.tile([128, QC], F32, tag="acc2", bufs=4,
                                        name=f"acc_{j}")
                                for j in range(HPC))

                    def emit_pv(j, t, pt):
                        base = (t * HPC + j) * 128
                        seen[j] += 1
                        nc.tensor.matmul(
                            accs2[j][:], t_v1[:, base:base + 128], pt[:],
                            start=(seen[j] == 1), stop=(seen[j] == trips[j]))
                        if seen[j] == trips[j]:
                            # normalize right after the slot's last PV:
                            # denominator is pre-broadcast in rows 64:127
                            p, half = j // 2, j % 2
                            rows = slice(half * 64, (half + 1) * 64)
                            den = nrmpool.tile([64, QC], F32, tag="den")
                            nc.vector.tensor_copy(
                                den[:], accs2[j][64:128, :])
                            rcp = nrmpool.tile([64, QC], F32, tag="rcp")
                            nc.vector.reciprocal_approx_fast(rcp[:], den[:])
                            with nc.allow_low_precision(reason="f32r 4B"):
                                nc.vector.tensor_mul(
                                    t_pb[p][rows, qs], accs2[j][0:64, :],
                                    rcp[:])

                    # chunk 0 carries the 16 V-projection c-steps spread
                    # over its first ~2/3 items (PVs defer until V's PSUM
                    # banks are free); later chunks carry the previous
                    # chunk's output projection instead
                    vsched, op_sched = {}, {}
                    if q == 0:
                        span = max(32, int(nitems * 0.75))
                        for s in range(32):
                            vsched.setdefault(
                                min(1 + s * span // 32, nitems - 1), []
                            ).append(s)
                        vleft = 32
                    else:
                        vleft = 0
                        step = max(1, (nitems - 4) // 4)
                        for k in range(4):
                            op_sched[min(2 + k * step, nitems - 1)] = k

                    k = 0
                    while k < nitems:
                        batch = items[k:k + 2]
                        sts = []
                        for (j, t) in batch:
                            p, half = j // 2, j % 2
                            rows = slice(half * 64, (half + 1) * 64)
                            sT = ap.tile([128, QC], F32, tag="sT", bufs=4)
                            nc.tensor.matmul(
                                sT[:], t_kT[p][rows, t * 128:(t + 1) * 128],
                                t_qT[p][rows, qs], start=True, stop=True)
                            sts.append(sT)
                        for (j, t), sT in zip(batch, sts):
                            pT = ptpool.tile([128, QC], BF16, tag="pT")
                            nc.scalar.activation(
                                pT[:], sT[:], AF.Exp, scale=0.125,
                                bias=t_vm[:, j * NKT + t: j * NKT + t + 1])
                            pend.append((j, t, pT))
                        for i in (k, k + 1):
                            for s in vsched.get(i, ()):
                                v_step(s // 8, s % 8)
                                vleft -= 1
                        if vleft == 0:
                            ensure_accs2()
                            budget = 4 if q == 0 else 2
                            while len(pend) > LOOKAHEAD and budget:
                                emit_pv(*pend.popleft())
                                budget -= 1
                        for i in (k, k + 1):
                            if i in op_sched:
                                emit_outproj_qt((q - 1) * 4 + op_sched[i])
                        k += 2
                    ensure_accs2()
                    while pend:
                        emit_pv(*pend.popleft())
                # last chunk's output projection
                for qt in range((N // QC - 1) * 4, (N // QC) * 4):
                    emit_outproj_qt(qt)

    nc.finalize()
    return nc


def kernel(queries, keys, values, valid_len, Wq, Wk, Wv, Wo):
    global LAST_RESULTS
    queries = np.asarray(queries, dtype=np.float32)
    keys = np.asarray(keys, dtype=np.float32)
    values = np.asarray(values, dtype=np.float32)
    Wq = np.asarray(Wq, dtype=np.float32)
    Wk = np.asarray(Wk, dtype=np.float32)
    Wv = np.asarray(Wv, dtype=np.float32)
    Wo = np.asarray(Wo, dtype=np.float32)
    vl = np.asarray(valid_len).astype(np.int64).reshape(B * H)

    # rank-aligned slot assignment: per batch, heads sorted by vl desc;
    # slot j of the 4 cores of that batch takes ranks 4j..4j+3
    order = {}
    for b in range(B):
        idx = (np.argsort(-vl[b * H:(b + 1) * H], kind="stable") + b * H)
        for cg in range(4):
            order[b * 4 + cg] = [int(idx[4 * j + cg]) for j in range(HPC)]
    trips = []
    for j in range(HPC):
        vs = [int(vl[order[c][j]]) for c in range(NCORES)]
        m = max(-(-v // 128) for v in vs)
        trips.append(max(1, min(NKT, m)))

    nc = _build_program(tuple(trips))

    in_maps = []
    for c in range(NCORES):
        b = c // 4
        heads = order[c]
        cols = np.concatenate(
            [np.arange((h - b * H) * DH, (h - b * H + 1) * DH) for h in heads])

        def wlayout(w):
            return np.ascontiguousarray(
                w[:, cols].reshape(NDC * 128, 256).astype(NPBF16))

        vm = np.zeros((128, HPC * NKT), np.float32)
        for j, h in enumerate(heads):
            bias = np.where(np.arange(N) < vl[h], 0.0, MASK_BIAS)
            vm[:, j * NKT:(j + 1) * NKT] = bias.reshape(NKT, 128).T

        in_maps.append({
            "xTq": np.ascontiguousarray(queries[b].T.astype(NPBF16)),
            "xTk": np.ascontiguousarray(keys[b].T.astype(NPBF16)),
            "xTv": np.ascontiguousarray(values[b].T.astype(NPBF16)),
            "wq": wlayout(Wq),
            "wk": wlayout(Wk),
            "wv": wlayout(Wv),
            "wo": np.ascontiguousarray(Wo[cols, :]).astype(NPBF16),
            "vmask": vm,
        })

    LAST_RESULTS = run_bass_kernel_spmd(nc, in_maps, list(range(NCORES)))
    res = LAST_RESULTS.results

    out = np.zeros((B, N, D), np.float64)
    for c in range(NCORES):
        out[c // 4] += res[c]["out"].astype(np.float64)
    return out.astype(np.float32)


# revision 15
# speedup vs baseline: 1.4946x; 1.0114x over previous
"""Multi-head attention TRN2 kernel (8 NeuronCores, SPMD).

Problem: B=2, N=2048, D=1024, H=16 heads of dim 64, fp32, per-(b,h)
key-length masking (valid_len, length 32).

Sharding: batch*heads across 8 cores - core c handles batch b=c//4 and 4
heads ("slots", rank-aligned by valid_len so the SPMD trip counts stay
balanced).  Per core:

  phase P (projections, bf16 inputs to halve HBM traffic):
    K^T/Q^T = Wslice^T @ x^T   (head dims on partitions, positions free)
    V       = x^T-tiles as lhsT, Wv as rhs  (positions on partitions),
              copied into V1 = [V_j | ones(64)] blocks per (key-tile, slot)
              - the 64 replicated ones columns make the PV matmul emit the
              softmax denominator PRE-BROADCAST in PSUM rows 64:127
  phase A (attention, all-bf16 operands, f32 PSUM accumulate):
    flat round-robin over (slot, key-tile) items per 512-query chunk with
    a PV lookahead stagger: S(i) and exp(i) are emitted immediately, but
    PV(i-L) is emitted L items later, so a PV waiting on its exp never
    head-of-line-blocks the next S matmul in the PE's strict-FIFO queue
    (the previous interleave serialized S->exp->PV per group, leaving
    both PE and ScalarE ~50% idle and oscillating the HAM clock gate)
    S^T   = K^T.T @ Q^T per (slot, key-tile), 1 PSUM bank, 4 rotating
    P^T   = exp(S^T/8 + bias) on ScalarE - valid_len mask is a
            per-partition bias column (0 / -30000)
    acc   = V1.T @ P^T accumulated over key tiles; rows 64:127 hold the
            denominator replicated across 64 partitions
    normalize per slot right after its last PV: reciprocal_approx_fast
    (DVE, ~5x faster than bit-exact) + one tensor_mul - no partition
    broadcast needed
    out_partial = heads^T.T @ Wo_slice, interleaved a few items into the
    NEXT chunk so its PSUM slots never starve the S-matmul pipeline
Host sums the 4 per-core partials of each batch element (the unshard for
the row-sharded Wo) and gathers.
"""
import sys
import numpy as np
from collections import deque
from contextlib import ExitStack

sys.path.insert(0, "/opt/trn_rl_repo")

import concourse.bass as bass  # noqa: E402
from concourse import bacc, mybir  # noqa: E402
import concourse.tile as tile  # noqa: E402
from concourse.bass_utils import run_bass_kernel_spmd  # noqa: E402

F32 = mybir.dt.float32
BF16 = mybir.dt.bfloat16
AF = mybir.ActivationFunctionType
NPBF16 = mybir.dt.np(BF16)

B, N, D, H = 2, 2048, 1024, 16
DH = 64
HPC = 4          # heads (slots) per core
NCORES = 8
QC = 512         # q chunk (matmul free dim)
NKT = N // 128   # 16 k tiles
NDC = D // 128   # 8 contraction chunks
MASK_BIAS = -30000.0
LOOKAHEAD = 3    # PV stagger (in items) behind S/exp emission

LAST_RESULTS = None  # BassKernelResults of the most recent run (for tooling)


def _build_program(trips):
    """trips: 4 ints (k-tile count per slot)."""
    nc = bacc.Bacc("TRN2", target_bir_lowering=False, debug=False,
                   num_devices=NCORES)

    xTq = nc.dram_tensor("xTq", [D, N], BF16, kind="ExternalInput")
    xTk = nc.dram_tensor("xTk", [D, N], BF16, kind="ExternalInput")
    xTv = nc.dram_tensor("xTv", [D, N], BF16, kind="ExternalInput")
    wq = nc.dram_tensor("wq", [NDC * 128, 256], BF16, kind="ExternalInput")
    wk = nc.dram_tensor("wk", [NDC * 128, 256], BF16, kind="ExternalInput")
    wv = nc.dram_tensor("wv", [NDC * 128, 256], BF16, kind="ExternalInput")
    wo = nc.dram_tensor("wo", [256, D], BF16, kind="ExternalInput")
    vmask = nc.dram_tensor("vmask", [128, HPC * NKT], F32, kind="ExternalInput")
    out = nc.dram_tensor("out", [N, D], BF16, kind="ExternalOutput")

    # flat item list per chunk: round-robin tiles across slots so adjacent
    # items hit different slots (independent chains)
    items = []
    for r in range(max(trips)):
        for j in range(HPC):
            if r < trips[j]:
                items.append((j, r))

    with tile.TileContext(nc) as tc:
        with ExitStack() as ctx:
            wpool = ctx.enter_context(tc.tile_pool(name="wpool", bufs=1))
            xpool = ctx.enter_context(tc.tile_pool(name="xpool", bufs=6))
            qkpool = ctx.enter_context(tc.tile_pool(name="qkpool", bufs=1))
            v1pool = ctx.enter_context(tc.tile_pool(name="v1pool", bufs=1))
            ptpool = ctx.enter_context(tc.tile_pool(name="ptpool", bufs=40))
            nrmpool = ctx.enter_context(tc.tile_pool(name="nrmpool", bufs=4))
            pbpool = ctx.enter_context(tc.tile_pool(name="pbpool", bufs=1))
            opool = ctx.enter_context(tc.tile_pool(name="opool", bufs=8))

            t_wk = wpool.tile([128, NDC * 256], BF16, tag="wk")
            t_wq = wpool.tile([128, NDC * 256], BF16, tag="wq")
            t_wv = wpool.tile([128, NDC * 256], BF16, tag="wv")
            t_wo = [wpool.tile([128, D], BF16, tag=f"wo{p}", name=f"t_wo{p}")
                    for p in range(2)]
            t_vm = wpool.tile([128, HPC * NKT], F32, tag="vm")

            # K^T/Q^T: [128 dims (2 slots), N] per slot-pair
            t_kT = [qkpool.tile([128, N], BF16, tag=f"kT{p}", name=f"t_kT{p}")
                    for p in range(2)]
            t_qT = [qkpool.tile([128, N], BF16, tag=f"qT{p}", name=f"t_qT{p}")
                    for p in range(2)]
            # V1: per key-tile t, 4 blocks of [V_j (64 cols) | ones (64 cols)]
            t_v1 = v1pool.tile([128, NKT * HPC * 128], BF16, tag="v1")
            # normalized heads^T per slot pair: [128 dims, N]
            t_pb = [pbpool.tile([128, N], BF16, tag=f"pb{p}", name=f"t_pb{p}")
                    for p in range(2)]
            # scratch for the ACT exp-table preload
            t_pre = wpool.tile([1, 1], F32, tag="pre")

            # ---- phase P: projections (K, Q, V) ----
            with tc.tile_pool(name="pp", bufs=8, space="PSUM") as pp:
                for si, (xin, wsb, dsts) in enumerate(
                        ((xTk, t_wk, t_kT), (xTq, t_wq, t_qT))):
                    accs = [pp.tile([128, QC], F32, tag="acc", name=f"acc_{i}")
                            for i in range(8)]
                    for c in range(NDC):
                        xt = xpool.tile([128, N], BF16, tag="xt")
                        if si == 0 and c == 0:
                            # split the gating first chunk across 4 DMA
                            # queues so the first matmul fires sooner
                            for sp in range(4):
                                cs = slice(sp * 512, (sp + 1) * 512)
                                nc.sync.dma_start(
                                    xt[:, cs], xin[0:128, cs])
                        else:
                            nc.sync.dma_start(
                                xt[:], xin[c * 128:(c + 1) * 128, :])
                        if si == 0:
                            # wk sliced per chunk: the first matmul only
                            # gates on 64KB of weights + one x chunk
                            nc.sync.dma_start(
                                t_wk[:, c * 256:(c + 1) * 256],
                                wk[c * 128:(c + 1) * 128, :])
                        if si == 0 and c == 0:
                            # ACT exp-table preload + V1 ones memset, behind
                            # the first input DMAs in trigger order so they
                            # don't delay the x stream
                            nc.scalar.activation(t_pre[:], t_pre[:], AF.Exp)
                            ones_ap = t_v1[:].rearrange(
                                "p (b c) -> p b c", c=128)[:, :, 64:128]
                            nc.vector.memset(ones_ap, 1.0)
                        for m in range(2):
                            for qq in range(4):
                                nc.tensor.matmul(
                                    accs[m * 4 + qq][:],
                                    wsb[:, c * 256 + m * 128:
                                        c * 256 + (m + 1) * 128],
                                    xt[:, qq * QC:(qq + 1) * QC],
                                    start=(c == 0), stop=(c == NDC - 1))
                    wnext, tnext = (wq, t_wq) if si == 0 else (wv, t_wv)
                    for c in range(NDC):
                        nc.sync.dma_start(
                            tnext[:, c * 256:(c + 1) * 256],
                            wnext[c * 128:(c + 1) * 128, :])
                    # drain PSUM->SBUF casts on BOTH ScalarE and DVE, q-chunk
                    # 0 first: the first attention S matmuls gate only on the
                    # chunk-0 casts, so phase A starts ~2.5us earlier
                    for n, i in enumerate((0, 4, 1, 5, 2, 6, 3, 7)):
                        dst = dsts[i // 4][:, (i % 4) * QC:(i % 4 + 1) * QC]
                        with nc.allow_low_precision(reason="f32r 4B"):
                            if n % 2 == 0:
                                nc.scalar.activation(dst, accs[i][:], AF.Copy)
                            else:
                                nc.vector.tensor_copy(dst, accs[i][:])
                nc.sync.dma_start(t_wo[0][:], wo[0:128, :])
                nc.sync.dma_start(t_wo[1][:], wo[128:256, :])
                nc.sync.dma_start(t_vm[:], vmask[:])

            # ---- phase A: attention with V projection streamed into chunk
            # 0 (fills PE bubbles while ScalarE chews chunk-0 exps) and
            # fused output projection ----
            with tc.tile_pool(name="ap", bufs=1, space="PSUM") as ap:
                # V projection, split into 16 c-steps interleaved with the
                # chunk-0 S/exp stream.  PSUM: the two 4-bank pass groups
                # come from the same rotation ("acc2" tag) that the
                # attention accumulators use later - acc2 tiles allocate
                # only after V is done, so 4 sT + 4 V banks = 8 exactly.
                vaccs = [None]

                def v_step(sp, c):
                    # sub-pass sp covers 4 key-tiles (bank-aligned [128,512]
                    # slots, V data in cols 0:256), streamed over 8 c-steps
                    g, h = sp // 2, sp % 2
                    if c == 0:
                        vaccs[0] = [ap.tile([128, 512], F32, tag="acc2",
                                            bufs=4, name=f"vacc{sp}_{i}")
                                    for i in range(4)]
                    xt = xpool.tile([128, 512], BF16, tag="xtv")
                    nc.sync.dma_start(
                        xt[:], xTv[c * 128:(c + 1) * 128,
                                   g * 1024 + h * 512:
                                   g * 1024 + (h + 1) * 512])
                    for k in range(4):
                        nc.tensor.matmul(
                            vaccs[0][k][:, 0:256],
                            xt[:, k * 128:(k + 1) * 128],
                            t_wv[:, c * 256:(c + 1) * 256],
                            start=(c == 0), stop=(c == NDC - 1))
                    if c == NDC - 1:
                        for k in range(4):
                            t = g * 8 + h * 4 + k
                            # [128, 4, 64] strided copy: slot j -> V1 block
                            src = vaccs[0][k][:, 0:256].rearrange(
                                "p (j c) -> p j c", c=64)
                            dst = t_v1[:, t * 512:(t + 1) * 512].rearrange(
                                "p (j c) -> p j c", c=128)[:, :, 0:64]
                            with nc.allow_low_precision(reason="f32r 4B"):
                                nc.vector.tensor_copy(dst, src)
                def emit_outproj_qt(qt):
                    ts = slice(qt * 128, (qt + 1) * 128)
                    stage = opool.tile([128, D], BF16, tag="ostage")
                    o_ps = [ap.tile([128, 512], F32, tag="sT", bufs=4,
                                    name=f"o_ps{ch}") for ch in range(2)]
                    for p2 in (1, 0):
                        for ch in range(2):
                            nc.tensor.matmul(
                                o_ps[ch][:], t_pb[p2][:, ts],
                                t_wo[p2][:, ch * 512:(ch + 1) * 512],
                                start=(p2 == 1), stop=(p2 == 0))
                    for ch in range(2):
                        with nc.allow_low_precision(reason="bf16 out"):
                            nc.vector.tensor_copy(
                                stage[:, ch * 512:(ch + 1) * 512],
                                o_ps[ch][:])
                    nc.sync.dma_start(out[ts, :], stage[:])

                nitems = len(items)
                for q in range(N // QC):
                    qs = slice(q * QC, (q + 1) * QC)
                    accs2 = []  # allocated lazily, after V frees its banks
                    pend = deque()
                    seen = [0] * HPC

                    def ensure_accs2():
                        if not accs2:
                            accs2.extend(
                                ap.tile([128, QC], F32, tag="acc2", bufs=4,
                                        name=f"acc_{j}")
                                for j in range(HPC))

                    def emit_pv(j, t, pt):
                        base = (t * HPC + j) * 128
                        seen[j] += 1
                        nc.tensor.matmul(
                            accs2[j][:], t_v1[:, base:base + 128], pt[:],
                            start=(seen[j] == 1), stop=(seen[j] == trips[j]))
                        if seen[j] == trips[j]:
                            # normalize right after the slot's last PV:
                            # denominator is pre-broadcast in rows 64:127
                            p, half = j // 2, j % 2
                            rows = slice(half * 64, (half + 1) * 64)
                            den = nrmpool.tile([64, QC], F32, tag="den")
                            nc.vector.tensor_copy(
                                den[:], accs2[j][64:128, :])
                            rcp = nrmpool.tile([64, QC], F32, tag="rcp")
                            nc.vector.reciprocal_approx_fast(rcp[:], den[:])
                            with nc.allow_low_precision(reason="f32r 4B"):
                                nc.vector.tensor_mul(
                                    t_pb[p][rows, qs], accs2[j][0:64, :],
                                    rcp[:])

                    # chunk 0 carries the 16 V-projection c-steps spread
                    # over its first ~2/3 items (PVs defer until V's PSUM
                    # banks are free); later chunks carry the previous
                    # chunk's output projection instead
                    vsched, op_sched = {}, {}
                    if q == 0:
                        span = max(32, int(nitems * 0.75))
                        for s in range(32):
                            vsched.setdefault(
                                min(1 + s * span // 32, nitems - 1), []
                            ).append(s)
                        vleft = 32
                    else:
                        vleft = 0
                        step = max(1, (nitems - 4) // 4)
                        for k in range(4):
                            op_sched[min(2 + k * step, nitems - 1)] = k

                    k = 0
                    while k < nitems:
                        batch = items[k:k + 2]
                        sts = []
                        for (j, t) in batch:
                            p, half = j // 2, j % 2
                            rows = slice(half * 64, (half + 1) * 64)
                            sT = ap.tile([128, QC], F32, tag="sT", bufs=4)
                            nc.tensor.matmul(
                                sT[:], t_kT[p][rows, t * 128:(t + 1) * 128],
                                t_qT[p][rows, qs], start=True, stop=True)
                            sts.append(sT)
                        for (j, t), sT in zip(batch, sts):
                            pT = ptpool.tile([128, QC], BF16, tag="pT")
                            nc.scalar.activation(
                                pT[:], sT[:], AF.Exp, scale=0.125,
                                bias=t_vm[:, j * NKT + t: j * NKT + t + 1])
                            pend.append((j, t, pT))
                        for i in (k, k + 1):
                            for s in vsched.get(i, ()):
                                v_step(s // 8, s % 8)
                                vleft -= 1
                        if vleft == 0:
                            ensure_accs2()
                            budget = 4 if q == 0 else 2
                            while len(pend) > LOOKAHEAD and budget:
                                emit_pv(*pend.popleft())
                                budget -= 1
                        for i in (k, k + 1):
                            if i in op_sched:
                                emit_outproj_qt((q - 1) * 4 + op_sched[i])
                        k += 2
                    ensure_accs2()
                    while pend:
                        emit_pv(*pend.popleft())
                # last chunk's output projection
                for qt in range((N // QC - 1) * 4, (N // QC) * 4):
                    emit_outproj_qt(qt)

    nc.finalize()
    return nc


def kernel(queries, keys, values, valid_len, Wq, Wk, Wv, Wo):
    global LAST_RESULTS
    queries = np.asarray(queries, dtype=np.float32)
    keys = np.asarray(keys, dtype=np.float32)
    values = np.asarray(values, dtype=np.float32)
    Wq = np.asarray(Wq, dtype=np.float32)
    Wk = np.asarray(Wk, dtype=np.float32)
    Wv = np.asarray(Wv, dtype=np.float32)
    Wo = np.asarray(Wo, dtype=np.float32)
    vl = np.asarray(valid_len).astype(np.int64).reshape(B * H)

    # rank-aligned slot assignment: per batch, heads sorted by vl desc;
    # slot j of the 4 cores of that batch takes ranks 4j..4j+3
    order = {}
    for b in range(B):
        idx = (np.argsort(-vl[b * H:(b + 1) * H], kind="stable") + b * H)
        for cg in range(4):
            order[b * 4 + cg] = [int(idx[4 * j + cg]) for j in range(HPC)]
    trips = []
    for j in range(HPC):
        vs = [int(vl[order[c][j]]) for c in range(NCORES)]
        m = max(-(-v // 128) for v in vs)
        trips.append(max(1, min(NKT, m)))

    nc = _build_program(tuple(trips))

    in_maps = []
    for c in range(NCORES):
        b = c // 4
        heads = order[c]
        cols = np.concatenate(
            [np.arange((h - b * H) * DH, (h - b * H + 1) * DH) for h in heads])

        def wlayout(w):
            return np.ascontiguousarray(
                w[:, cols].reshape(NDC * 128, 256).astype(NPBF16))

        vm = np.zeros((128, HPC * NKT), np.float32)
        for j, h in enumerate(heads):
            bias = np.where(np.arange(N) < vl[h], 0.0, MASK_BIAS)
            vm[:, j * NKT:(j + 1) * NKT] = bias.reshape(NKT, 128).T

        in_maps.append({
            "xTq": np.ascontiguousarray(queries[b].T.astype(NPBF16)),
            "xTk": np.ascontiguousarray(keys[b].T.astype(NPBF16)),
            "xTv": np.ascontiguousarray(values[b].T.astype(NPBF16)),
            "wq": wlayout(Wq),
            "wk": wlayout(Wk),
            "wv": wlayout(Wv),
            "wo": np.ascontiguousarray(Wo[cols, :]).astype(NPBF16),
            "vmask": vm,
        })

    LAST_RESULTS = run_bass_kernel_spmd(nc, in_maps, list(range(NCORES)))
    res = LAST_RESULTS.results

    out = np.zeros((B, N, D), np.float64)
    for c in range(NCORES):
        out[c // 4] += res[c]["out"].astype(np.float64)
    return out.astype(np.float32)
